# revision 6
# baseline (speedup 1.0000x reference)
"""Trainium2 Bass kernel for nn_LogicGatedSpikingSelfAttention.

Sharding: channel/head-parallel over 8 cores. Each core owns 128 output
channels = 2 heads for the q/k/v branches (BN stats fully local), runs
attention for its 2 heads over all 4 batches, and computes a 128-channel
slice of the projection over the gathered (pre-gated) attention spikes.

Pipeline structure (vs the v0 baseline):
- Input DMA ordered so the q branch streams kt-by-kt as x tiles land.
- BN stats read directly from PSUM so thresholds resolve earlier.
- A tiny energy AllGather fires mid-v-branch; gates are computed on the
  SENDER for its own 2 heads and folded into the attention-spike
  thresholds (gate is {0,1}: g*(S>=th) == S >= th + BIG*(1-g)), so the
  post-gather gate/weight-scaling stage disappears entirely.
- The payload AllGather is split into two 2-batch chunks issued as soon
  as each chunk's scores are thresholded; the projection consumes chunk
  0 while chunk 1 is still in flight.
- v spikes are transposed on the PE (identity matmul) instead of the
  DMA XBAR, removing ~4k tiny descriptors from the critical path.
- Projection PSUM drains to SBUF f32 (so attention psum and projection
  psum fit the 8 banks together); output spikes stored as fp8.
All arithmetic is bit-identical to v0: bf16 inputs, f32 psum
accumulation in the same kt/t order, same bn_stats/bn_aggr formulation.
"""
import numpy as np
import ml_dtypes

import concourse.bass as bass
import concourse.bass_isa as bass_isa
import concourse.bacc as bacc
import concourse.tile as tile
from concourse import mybir
from concourse.bass_utils import run_bass_kernel_spmd

NCORES = 8
B, NSEQ, D, H = 4, 1024, 1024, 16
HD = D // H            # 64 head dim
CH = D // NCORES       # 128 channels per core
TOK = B * NSEQ         # 4096 tokens
KT = D // 128          # 8 contraction tiles
EPS = 1e-5
S_TH = float(2.0 ** 0.75)   # x_attn >= 1  <=>  S >= hd**0.125 = 2^0.75
NCHUNK = 2                  # payload exchange chunks
CB = B // NCHUNK            # batches per chunk
CTOK = CB * NSEQ            # tokens per chunk
F32 = mybir.dt.float32
BF16 = mybir.dt.bfloat16
FP16 = mybir.dt.float16
FP8 = mybir.dt.float8e4
BF = ml_dtypes.bfloat16
AF = mybir.ActivationFunctionType
OP = mybir.AluOpType

_CACHE = {}


def _build():
    nc = bacc.Bacc("TRN2", target_bir_lowering=False, debug=False,
                   num_devices=NCORES)
    inp = {}
    def din(name, shape, dt=BF16):
        inp[name] = nc.dram_tensor(name, shape, dt, kind="ExternalInput")
        return inp[name]

    din("xT",  [128, KT * TOK])          # host pre-tiled: [p, (t n)]
    din("wq",  [128, KT * CH]); din("wk", [128, KT * CH])
    din("wv",  [128, KT * CH]); din("wp", [128, KT * CH])
    for nm in ("tq", "tk", "tv", "tp"):
        din(nm, [CH, 1], F32)
    din("wgr2", [H, 2], F32)             # lhsT cols for this core's 2 heads
    din("bgr2", [2, 1], F32)
    din("i2e", [CH, 2], F32)             # [p, j] = (p//64==j)
    din("i2eT", [2, CH], F32)            # transpose of i2e
    din("idn", [128, 128], FP16)         # identity for PE transposes
    outT = nc.dram_tensor("outT", [128, TOK], FP8, kind="ExternalOutput")

    with tile.TileContext(nc) as tc:
        with tc.tile_pool(name="consts", bufs=1) as consts, \
             tc.tile_pool(name="spikes", bufs=1) as spk, \
             tc.tile_pool(name="dram", bufs=1, space="DRAM") as dram:
            _body(tc, inp, outT, consts, spk, dram)
    nc.compile()
    return nc


def _body(tc, inp, outT, consts, spk, dram):
    nc = tc.nc
    V, SC, GP, TE = nc.vector, nc.scalar, nc.gpsimd, nc.tensor
    DENG = [nc.sync, nc.scalar, nc.gpsimd]

    # ---- small consts ----
    small = {}
    for nm in ("tq", "tk", "tv", "tp", "bgr2"):
        t = consts.tile([inp[nm].shape[0], 1], F32, name=f"{nm}_sb")
        small[nm] = t
    wgr2_sb = consts.tile([H, 2], F32)
    i2e_sb = consts.tile([CH, 2], F32)
    i2eT_sb = consts.tile([2, CH], F32)
    idn_sb = consts.tile([128, 128], FP16)
    eps_sb = consts.tile([128, 1], F32)
    V.memset(eps_sb[:], EPS)
    w_sb = {nm: consts.tile([128, KT, CH], BF16, name=f"{nm}_sb")
            for nm in ("wq", "wk", "wv", "wp")}

    # ---- DRAM staging for collectives ----
    e_pay = dram.tile([2 * B], F32)
    e_gath = dram.tile([NCORES, 2 * B], F32, addr_space="Shared")
    pay_d = [dram.tile([128 * CTOK], FP8, name=f"pay{c}")
             for c in range(NCHUNK)]
    gath_d = [dram.tile([NCORES, 128 * CTOK], FP8, addr_space="Shared",
                        name=f"gath{c}") for c in range(NCHUNK)]

    # ---- persistent tensors (live across phase pools) ----
    e_sb = spk.tile([2, B], F32)
    ech = spk.tile([128, B], F32)
    spA = {nm: spk.tile([128, TOK], FP16, name=f"sp{nm}A")
           for nm in ("q", "k", "v")}
    ktok = spk.tile([128, B * 8, 128], FP16)   # [tok, blk, ch] via XBAR
    payload = spk.tile([128, TOK], FP8)        # gated attention spikes
    prod = spk.tile([128, TOK], FP16)
    thrb = spk.tile([128, B], F32)             # gated S thresholds per batch
    biasb = spk.tile([128, B], F32)            # gated sigmoid bias per batch
    Yv = spk.tile([128, TOK], F32)             # v linear output (f32)
    thr_v = spk.tile([128, 1], F32)
    projY = spk.tile([128, TOK], F32)
    osb = spk.tile([128, TOK], FP8)
    pstats = spk.tile([128, 8, 6], F32)

    # ================= branches (q, k, v) =================
    # Linear bias cancels inside BatchNorm. BN stats are computed straight
    # from PSUM as each bank stops; drains (SC/V split) free banks for the
    # next branch; spikes then compare the SBUF f32 copy to the threshold.
    with tc.tile_pool(name="xtp", bufs=1) as xtp, \
         tc.tile_pool(name="ybig", bufs=2) as ybig, \
         tc.tile_pool(name="stps", bufs=2) as stp:
        # loads ordered by first consumption: wq, then x tiles in kt order
        xre = inp["xT"].ap().rearrange("p (t n) -> p t n", t=KT)
        wre = {nm: inp[nm].ap().rearrange("p (t m) -> p t m", t=KT)
               for nm in ("wq", "wk", "wv", "wp")}
        xts = [xtp.tile([128, TOK], BF16, name=f"xts{kt}")
               for kt in range(KT)]
        nc.scalar.dma_start(w_sb["wq"][:], wre["wq"])
        nc.sync.dma_start(xts[0][:], xre[:, 0, :])
        nc.gpsimd.dma_start(xts[1][:], xre[:, 1, :])
        nc.scalar.dma_start(xts[2][:], xre[:, 2, :])
        nc.sync.dma_start(xts[3][:], xre[:, 3, :])
        nc.gpsimd.dma_start(xts[4][:], xre[:, 4, :])
        nc.scalar.dma_start(xts[5][:], xre[:, 5, :])
        nc.sync.dma_start(xts[6][:], xre[:, 6, :])
        nc.gpsimd.dma_start(xts[7][:], xre[:, 7, :])
        nc.scalar.dma_start(w_sb["wk"][:], wre["wk"])
        nc.gpsimd.dma_start(w_sb["wv"][:], wre["wv"])
        nc.scalar.dma_start(w_sb["wp"][:], wre["wp"])
        for nm in ("tq", "tk", "tv", "tp", "bgr2"):
            nc.sync.dma_start(small[nm][:], inp[nm].ap())
        nc.sync.dma_start(wgr2_sb[:], inp["wgr2"].ap())
        nc.sync.dma_start(i2e_sb[:], inp["i2e"].ap())
        nc.sync.dma_start(i2eT_sb[:], inp["i2eT"].ap())
        nc.gpsimd.dma_start(idn_sb[:], inp["idn"].ap())

        for nm in ("q", "k", "v"):
            Y = Yv if nm == "v" else ybig.tile([128, TOK], F32, tag="Y")
            stats = stp.tile([128, 8, 6], F32, tag="stats")
            if nm == "v":
                # two 4-bank phases; the spare banks run the energy
                # head-sum + gate matmuls so the tiny energy AllGather
                # fires mid-v-branch (gates resolve before scores).
                with tc.tile_pool(name="enps", bufs=2, space="PSUM") as enps, \
                     tc.tile_pool(name="brps_v", bufs=1,
                                  space="PSUM") as brps:
                    ps = [brps.tile([128, 512], F32, name=f"psv{i}")
                          for i in range(4)]
                    for half in range(2):
                        for kt in range(KT):
                            for j in range(4):
                                i = 4 * half + j
                                TE.matmul(
                                    ps[j][:], w_sb["wv"][:, kt, :],
                                    xts[kt][:, i * 512:(i + 1) * 512],
                                    start=(kt == 0), stop=(kt == KT - 1))
                        if half == 0:
                            gt = enps.tile([128, B], F32, tag="gt")
                            TE.matmul(gt[0:2, :], i2e_sb[:], ech[:],
                                      start=True, stop=True)
                            V.tensor_copy(e_sb[:], gt[0:2, :])
                            nc.scalar.dma_start(
                                e_pay[:].rearrange("(p w) -> p w", p=2),
                                e_sb[:])
                            nc.gpsimd.collective_compute(
                                "AllGather", OP.bypass,
                                ins=[e_pay.opt()], outs=[e_gath.opt()],
                                replica_groups=[list(range(NCORES))])
                        for j in range(4):
                            i = 4 * half + j
                            V.bn_stats(stats[:, i, :], ps[j][:])
                            if j % 2:
                                V.tensor_copy(Y[:, i * 512:(i + 1) * 512],
                                              ps[j][:])
                            else:
                                SC.activation(Y[:, i * 512:(i + 1) * 512],
                                              ps[j][:], AF.Copy)
                    # gate math: e_gath -> own-2-head gates -> per-batch
                    # gated thresholds (exact: gate in {0,1})
                    e_all = stp.tile([H, B], F32, tag="eall")
                    nc.sync.dma_start(
                        e_all[:],
                        e_gath[:, :].rearrange("c (p w) -> (c p) w", p=2))
                    gt2 = enps.tile([128, B], F32, tag="gt")
                    TE.matmul(gt2[0:2, :], wgr2_sb[:], e_all[:],
                              start=True, stop=True)
                    gate2 = stp.tile([2, B], F32, tag="gate2")
                    V.tensor_scalar(gate2[:], gt2[0:2, :], small["bgr2"][:],
                                    0.5, OP.add, OP.is_ge)
                    gt3 = enps.tile([128, B], F32, tag="gt")
                    TE.matmul(gt3[:], i2eT_sb[:], gate2[:],
                              start=True, stop=True)
                    g128 = stp.tile([128, B], F32, tag="g128sb")
                    V.tensor_copy(g128[:], gt3[:])
                    V.tensor_scalar(thrb[:], g128[:], -2000.0,
                                    S_TH + 2000.0, OP.mult, OP.add)
                    V.tensor_scalar(biasb[:], g128[:], 1.0e6,
                                    -(1.0e6 + 128.0 * S_TH),
                                    OP.mult, OP.add)
            else:
                # weight-stationary: kt outer, 8 PSUM banks accumulate
                with tc.tile_pool(name=f"brps_{nm}", bufs=1,
                                  space="PSUM") as brps:
                    ps = [brps.tile([128, 512], F32, name=f"ps{nm}{i}")
                          for i in range(8)]
                    for kt in range(KT):
                        for nck in range(8):
                            TE.matmul(ps[nck][:], w_sb["w" + nm][:, kt, :],
                                      xts[kt][:, nck * 512:(nck + 1) * 512],
                                      start=(kt == 0), stop=(kt == KT - 1))
                    for i in range(8):
                        V.bn_stats(stats[:, i, :], ps[i][:])
                        if i % 2:
                            V.tensor_copy(Y[:, i * 512:(i + 1) * 512],
                                          ps[i][:])
                        else:
                            SC.activation(Y[:, i * 512:(i + 1) * 512],
                                          ps[i][:], AF.Copy)
            mv = stp.tile([128, 2], F32, tag="mv")
            V.bn_aggr(mv[:], stats[:])
            std = stp.tile([128, 1], F32, tag="std")
            SC.activation(std[:], mv[:, 1:2], AF.Sqrt, bias=eps_sb[:])
            thr = thr_v if nm == "v" else stp.tile([128, 1], F32,
                                                   tag=f"thr_{nm}")
            V.tensor_tensor(thr[:], std[:], small["t" + nm][:], OP.mult)
            V.tensor_tensor(thr[:], thr[:], mv[:, 0:1], OP.add)
            if nm == "v":
                continue   # v spikes happen per-batch in the attention loop
            for i in range(8):
                eng = V if i % 2 else GP
                eng.tensor_scalar(spA[nm][:, i * 512:(i + 1) * 512],
                                  Y[:, i * 512:(i + 1) * 512],
                                  thr[:], None, OP.is_ge)
            if nm == "k":
                for b in range(B):
                    [nc.sync, nc.scalar][b % 2].dma_start_transpose(
                        ktok[:, 8 * b:8 * b + 8, :],
                        spA["k"][:, b * NSEQ:(b + 1) * NSEQ])
                # energy elementwise part (overlaps v branch)
                GP.tensor_tensor(prod[:], spA["q"][:], spA["k"][:], OP.mult)
                V.reduce_sum(ech[:],
                             prod[:].rearrange("p (b n) -> p b n", b=B),
                             axis=mybir.AxisListType.X)

    # ============ attention + chunked exchange + projection ============
    # Per batch: v spikes -> PE transpose -> KV -> S^T -> gated threshold.
    # After each 2-batch chunk the payload AllGather is issued; the
    # projection for chunk c overlaps the AllGather of chunk c+1.
    with tc.tile_pool(name="atps", bufs=1, space="PSUM") as atps, \
         tc.tile_pool(name="s2ps", bufs=2, space="PSUM") as s2ps, \
         tc.tile_pool(name="tpps", bufs=1, space="PSUM") as tpps, \
         tc.tile_pool(name="ppps", bufs=3, space="PSUM") as ppps, \
         tc.tile_pool(name="kvsb", bufs=2) as kvsb, \
         tc.tile_pool(name="rhsp", bufs=2) as rhsp, \
         tc.tile_pool(name="pfin", bufs=1) as pfin:
        for b in range(B):
            n0b = b * NSEQ
            # v spikes for this batch (V/GP split), from the f32 copy
            V.tensor_scalar(spA["v"][:, n0b:n0b + 512],
                            Yv[:, n0b:n0b + 512], thr_v[:], None, OP.is_ge)
            GP.tensor_scalar(spA["v"][:, n0b + 512:n0b + NSEQ],
                             Yv[:, n0b + 512:n0b + NSEQ], thr_v[:],
                             None, OP.is_ge)
            # PE transpose to token-major (fp16, one psum bank)
            tp_ps = tpps.tile([128, 8, 128], FP16, tag="tp")
            for j in range(8):
                TE.transpose(tp_ps[:, j, :],
                             spA["v"][:, n0b + j * 128:n0b + (j + 1) * 128],
                             idn_sb[:])
            vtok_b = kvsb.tile([128, 8, 128], FP16, tag="vtok")
            V.tensor_copy(vtok_b[:], tp_ps[:])
            # KV[b] = k_tok^T @ v_tok, heads packed in PE columns
            kv_ps = atps.tile([128, HD], F32, tag="kvps")
            for mt in range(8):
                blk = b * 8 + mt
                TE.matmul(kv_ps[0:HD, :], ktok[:, blk, 0:HD],
                          vtok_b[:, mt, 0:HD],
                          start=(mt == 0), stop=(mt == 7),
                          tile_position=(0, 0))
                TE.matmul(kv_ps[HD:128, :], ktok[:, blk, HD:128],
                          vtok_b[:, mt, HD:128],
                          start=(mt == 0), stop=(mt == 7),
                          tile_position=(0, HD))
            kv = kvsb.tile([128, HD], FP16, tag="kv")
            if b % 2:
                V.tensor_copy(kv[:], kv_ps[:])
            else:
                SC.activation(kv[:], kv_ps[:], AF.Copy)
            # S^T = KV^T @ q with gated thresholds (scores are integers;
            # saturated sigmoid stays exact, gate folds into bias/thr)
            for ncn in range(2):
                n0 = n0b + ncn * 512
                s2 = s2ps.tile([128, 512], F32, tag="s2")
                TE.matmul(s2[0:HD, :], kv[0:HD, :],
                          spA["q"][0:HD, n0:n0 + 512],
                          start=True, stop=True, tile_position=(0, 0))
                TE.matmul(s2[HD:128, :], kv[HD:128, :],
                          spA["q"][HD:128, n0:n0 + 512],
                          start=True, stop=True, tile_position=(HD, HD))
                if ncn:
                    V.tensor_scalar(payload[:, n0:n0 + 512], s2[:],
                                    thrb[:, b:b + 1], None, OP.is_ge)
                else:
                    SC.activation(payload[:, n0:n0 + 512], s2[:],
                                  AF.Sigmoid, scale=128.0,
                                  bias=biasb[:, b:b + 1])
            if b % CB == CB - 1:
                c = b // CB
                DENG[c % 3].dma_start(
                    pay_d[c][:].rearrange("(p n) -> p n", p=128),
                    payload[:, c * CTOK:(c + 1) * CTOK])
                nc.gpsimd.collective_compute(
                    "AllGather", OP.bypass,
                    ins=[pay_d[c].opt()], outs=[gath_d[c].opt()],
                    replica_groups=[list(range(NCORES))])
                # projection for the previous chunk overlaps this gather
                if c > 0:
                    _proj_chunk(tc, c - 1, gath_d, w_sb, ppps, rhsp,
                                projY, pstats, DENG)
        _proj_chunk(tc, NCHUNK - 1, gath_d, w_sb, ppps, rhsp,
                    projY, pstats, DENG)

        # ---- final BN + spike threshold + store (fp8 {0,1} exact) ----
        mv = pfin.tile([128, 2], F32)
        V.bn_aggr(mv[:], pstats[:])
        std = pfin.tile([128, 1], F32)
        SC.activation(std[:], mv[:, 1:2], AF.Sqrt, bias=eps_sb[:])
        thr = pfin.tile([128, 1], F32)
        V.tensor_tensor(thr[:], std[:], small["tp"][:], OP.mult)
        V.tensor_tensor(thr[:], thr[:], mv[:, 0:1], OP.add)
        for i in range(8):
            V.tensor_scalar(osb[:, i * 512:(i + 1) * 512],
                            projY[:, i * 512:(i + 1) * 512],
                            thr[:], None, OP.is_ge)
        for b in range(B):
            DENG[b % 3].dma_start(
                outT.ap().rearrange("p (b n) -> p b n", b=B)[:, b, :],
                osb[:, b * NSEQ:(b + 1) * NSEQ])


def _proj_chunk(tc, c, gath_d, w_sb, ppps, rhsp, projY, pstats, DENG):
    """Projection matmuls for chunk c (CB batches), plain wp (payload is
    pre-gated). Accumulation order over t matches v0 exactly. PSUM banks
    drain to projY f32 so they cycle; BN stats stream from PSUM."""
    nc = tc.nc
    V, SC, TE = nc.vector, nc.scalar, nc.tensor
    rhs = rhsp.tile([128, KT, CTOK], FP8, tag="rhs")
    for t in range(KT):
        DENG[t % 3].dma_start(
            rhs[:, t, :],
            gath_d[c][t, :].rearrange("(p n) -> p n", p=128))
    for bl in range(CB):
        b = c * CB + bl
        for ncn in range(2):
            i = 2 * b + ncn
            ppt = ppps.tile([128, 512], F32, tag="ppt")
            for t in range(KT):
                TE.matmul(ppt[:], w_sb["wp"][:, t, :],
                          rhs[:, t, bl * NSEQ + ncn * 512:
                              bl * NSEQ + (ncn + 1) * 512],
                          start=(t == 0), stop=(t == KT - 1))
            V.bn_stats(pstats[:, i, :], ppt[:])
            if i % 2:
                V.tensor_copy(projY[:, i * 512:(i + 1) * 512], ppt[:])
            else:
                SC.activation(projY[:, i * 512:(i + 1) * 512], ppt[:],
                              AF.Copy)


def _tile_rows(a):
    # (8*128, N) -> (128, 8*N) so the SBUF [p, (t n)] load is contiguous
    n = a.shape[1]
    return np.ascontiguousarray(
        a.reshape(KT, 128, n).transpose(1, 0, 2).reshape(128, KT * n))


def _prep_inputs(inputs):
    x = np.asarray(inputs["x"], np.float32)
    xT = _tile_rows(x.reshape(TOK, D).T.astype(BF))
    Wg = np.asarray(inputs["Wg"], np.float64)
    wgr = (Wg.reshape(H, HD, H).sum(axis=1).T / 1024.0).astype(np.float32)
    bg = np.asarray(inputs["bg"], np.float32)
    i2e = np.zeros((CH, 2), np.float32)
    i2e[0:HD, 0] = 1.0
    i2e[HD:CH, 1] = 1.0
    i2eT = np.ascontiguousarray(i2e.T)
    idn = np.eye(128, dtype=np.float16)
    in_maps = []
    for c in range(NCORES):
        sl = slice(CH * c, CH * c + CH)
        m = {"xT": xT, "i2e": i2e, "i2eT": i2eT, "idn": idn,
             "wgr2": np.ascontiguousarray(wgr[:, 2 * c:2 * c + 2]),
             "bgr2": np.ascontiguousarray(bg[2 * c:2 * c + 2].reshape(2, 1))}
        for nm in ("q", "k", "v", "p"):
            W = np.asarray(inputs[f"W{nm}"], np.float32)
            m["w" + nm] = _tile_rows(W[sl, :].T.astype(BF))
            g = np.asarray(inputs[f"g{nm}"], np.float32)[sl]
            be = np.asarray(inputs[f"beta{nm}"], np.float32)[sl]
            m["t" + nm] = ((2.0 - be) / g).reshape(CH, 1).astype(np.float32)
        in_maps.append(m)
    return in_maps


def _run(inputs, trace=False):
    if "nc" not in _CACHE:
        _CACHE["nc"] = _build()
    nc = _CACHE["nc"]
    in_maps = _prep_inputs(inputs)
    res = run_bass_kernel_spmd(nc, in_maps, core_ids=list(range(NCORES)),
                               trace=trace)
    out = np.empty((TOK, D), np.float32)
    for c in range(NCORES):
        out[:, CH * c:CH * c + CH] = res.results[c]["outT"].astype(np.float32).T
    return out.reshape(B, NSEQ, D), res


def kernel(**inputs) -> np.ndarray:
    out, _ = _run(inputs, trace=False)
    return out


# revision 12
# speedup vs baseline: 1.2804x; 1.2804x over previous
"""Trainium2 Bass kernel for nn_LogicGatedSpikingSelfAttention.

Sharding: channel/head-parallel over 8 cores. Each core owns 128 output
channels = 2 heads for the q/k/v branches (BN stats fully local), runs
attention for its 2 heads over all 4 batches, and computes a 128-channel
slice of the projection over the gathered (pre-gated) attention spikes.

Pipeline structure (vs the v0 baseline):
- Input DMA ordered so the q branch streams kt-by-kt as x tiles land.
- BN stats read directly from PSUM so thresholds resolve earlier.
- A tiny energy AllGather fires mid-v-branch; gates are computed on the
  SENDER for its own 2 heads and folded into the attention-spike
  thresholds (gate is {0,1}: g*(S>=th) == S >= th + BIG*(1-g)), so the
  post-gather gate/weight-scaling stage disappears entirely.
- The payload AllGather is split into two 2-batch chunks issued as soon
  as each chunk's scores are thresholded; the projection consumes chunk
  0 while chunk 1 is still in flight.
- v spikes are transposed on the PE (identity matmul) instead of the
  DMA XBAR, removing ~4k tiny descriptors from the critical path.
- Projection PSUM drains to SBUF f32 (so attention psum and projection
  psum fit the 8 banks together); output spikes stored as fp8.
All arithmetic is bit-identical to v0: bf16 inputs, f32 psum
accumulation in the same kt/t order, same bn_stats/bn_aggr formulation.
"""
import numpy as np
import ml_dtypes

import concourse.bass as bass
import concourse.bass_isa as bass_isa
import concourse.bacc as bacc
import concourse.tile as tile
from concourse import mybir
from concourse.bass_utils import run_bass_kernel_spmd

NCORES = 8
B, NSEQ, D, H = 4, 1024, 1024, 16
HD = D // H            # 64 head dim
CH = D // NCORES       # 128 channels per core
TOK = B * NSEQ         # 4096 tokens
KT = D // 128          # 8 contraction tiles
EPS = 1e-5
S_TH = float(2.0 ** 0.75)   # x_attn >= 1  <=>  S >= hd**0.125 = 2^0.75
NCHUNK = 2                  # payload exchange chunks
CB = B // NCHUNK            # batches per chunk
CTOK = CB * NSEQ            # tokens per chunk
F32 = mybir.dt.float32
BF16 = mybir.dt.bfloat16
FP16 = mybir.dt.float16
FP8 = mybir.dt.float8e4
BF = ml_dtypes.bfloat16
AF = mybir.ActivationFunctionType
OP = mybir.AluOpType

_CACHE = {}


def _build():
    nc = bacc.Bacc("TRN2", target_bir_lowering=False, debug=False,
                   num_devices=NCORES)
    inp = {}
    def din(name, shape, dt=BF16):
        inp[name] = nc.dram_tensor(name, shape, dt, kind="ExternalInput")
        return inp[name]

    din("xT",  [128, KT * TOK])          # host pre-tiled: [p, (t n)]
    din("wq",  [128, KT * CH]); din("wk", [128, KT * CH])
    din("wv",  [128, KT * CH]); din("wp", [128, KT * CH])
    for nm in ("tq", "tk", "tv", "tp"):
        din(nm, [CH, 1], F32)
    din("wgr2", [H, 2], F32)             # lhsT cols for this core's 2 heads
    din("bgr2", [2, 1], F32)
    din("i2e", [CH, 2], F32)             # [p, j] = (p//64==j)
    din("i2eT", [2, CH], F32)            # transpose of i2e
    din("idn", [128, 128], FP16)         # identity for PE transposes
    outT = nc.dram_tensor("outT", [128, TOK], FP8, kind="ExternalOutput")

    with tile.TileContext(nc) as tc:
        with tc.tile_pool(name="consts", bufs=1) as consts, \
             tc.tile_pool(name="spikes", bufs=1) as spk, \
             tc.tile_pool(name="dram", bufs=1, space="DRAM") as dram:
            _body(tc, inp, outT, consts, spk, dram)
    nc.compile()
    return nc


def _body(tc, inp, outT, consts, spk, dram):
    nc = tc.nc
    V, SC, GP, TE = nc.vector, nc.scalar, nc.gpsimd, nc.tensor
    DENG = [nc.sync, nc.scalar, nc.gpsimd]

    # ---- small consts ----
    small = {}
    for nm in ("tq", "tk", "tv", "tp", "bgr2"):
        t = consts.tile([inp[nm].shape[0], 1], F32, name=f"{nm}_sb")
        small[nm] = t
    wgr2_sb = consts.tile([H, 2], F32)
    i2e_sb = consts.tile([CH, 2], F32)
    i2eT_sb = consts.tile([2, CH], F32)
    idn_sb = consts.tile([128, 128], FP16)
    eps_sb = consts.tile([128, 1], F32)
    V.memset(eps_sb[:], EPS)
    w_sb = {nm: consts.tile([128, KT, CH], BF16, name=f"{nm}_sb")
            for nm in ("wq", "wk", "wv", "wp")}

    # ---- DRAM staging for collectives ----
    e_pay = dram.tile([2 * B], F32)
    e_gath = dram.tile([NCORES, 2 * B], F32, addr_space="Shared")
    pay_d = dram.tile([128 * TOK], FP8)
    gath_d = dram.tile([NCORES, 128 * TOK], FP8, addr_space="Shared")

    # ---- persistent tensors (live across phase pools) ----
    e_sb = spk.tile([2, B], F32)
    ech = spk.tile([128, B], F32)
    spA = {nm: spk.tile([128, TOK], FP16, name=f"sp{nm}A")
           for nm in ("q", "k", "v")}
    ktok = spk.tile([128, B * 8, 128], FP16)   # [tok, blk, ch] via XBAR
    payload = spk.tile([128, TOK], FP8)        # gated attention spikes
    prod = spk.tile([128, TOK], FP16)
    thrb = spk.tile([128, B], F32)             # gated S thresholds per batch
    biasb = spk.tile([128, B], F32)            # gated sigmoid bias per batch
    Yv = spk.tile([128, TOK], F32)             # v linear output (f32)
    thr_v = spk.tile([128, 1], F32)
    osb = spk.tile([128, TOK], FP8)
    pstats = spk.tile([128, 8, 6], F32)

    # ================= branches (q, k, v) =================
    # Linear bias cancels inside BatchNorm. BN stats are computed straight
    # from PSUM as each bank stops; drains (SC/V split) free banks for the
    # next branch; spikes then compare the SBUF f32 copy to the threshold.
    with tc.tile_pool(name="xtp", bufs=1) as xtp, \
         tc.tile_pool(name="ybig", bufs=2) as ybig, \
         tc.tile_pool(name="stps", bufs=2) as stp:
        # loads ordered by first consumption: wq, then x tiles in kt order
        xre = inp["xT"].ap().rearrange("p (t n) -> p t n", t=KT)
        wre = {nm: inp[nm].ap().rearrange("p (t m) -> p t m", t=KT)
               for nm in ("wq", "wk", "wv", "wp")}
        xts = [xtp.tile([128, TOK], BF16, name=f"xts{kt}")
               for kt in range(KT)]
        nc.scalar.dma_start(w_sb["wq"][:], wre["wq"])
        nc.sync.dma_start(xts[0][:], xre[:, 0, :])
        nc.gpsimd.dma_start(xts[1][:], xre[:, 1, :])
        nc.scalar.dma_start(xts[2][:], xre[:, 2, :])
        nc.sync.dma_start(xts[3][:], xre[:, 3, :])
        nc.gpsimd.dma_start(xts[4][:], xre[:, 4, :])
        nc.scalar.dma_start(xts[5][:], xre[:, 5, :])
        nc.sync.dma_start(xts[6][:], xre[:, 6, :])
        nc.gpsimd.dma_start(xts[7][:], xre[:, 7, :])
        nc.scalar.dma_start(w_sb["wk"][:], wre["wk"])
        nc.gpsimd.dma_start(w_sb["wv"][:], wre["wv"])
        nc.scalar.dma_start(w_sb["wp"][:], wre["wp"])
        for nm in ("tq", "tk", "tv", "tp", "bgr2"):
            nc.sync.dma_start(small[nm][:], inp[nm].ap())
        nc.sync.dma_start(wgr2_sb[:], inp["wgr2"].ap())
        nc.sync.dma_start(i2e_sb[:], inp["i2e"].ap())
        nc.sync.dma_start(i2eT_sb[:], inp["i2eT"].ap())
        nc.gpsimd.dma_start(idn_sb[:], inp["idn"].ap())

        for nm in ("q", "k", "v"):
            Y = Yv if nm == "v" else ybig.tile([128, TOK], F32, tag="Y")
            stats = stp.tile([128, 8, 6], F32, tag="stats")
            if nm == "v":
                # two 4-bank phases; the spare banks run the energy
                # head-sum + gate matmuls so the tiny energy AllGather
                # fires mid-v-branch (gates resolve before scores).
                with tc.tile_pool(name="enps", bufs=2, space="PSUM") as enps, \
                     tc.tile_pool(name="brps_v", bufs=1,
                                  space="PSUM") as brps:
                    ps = [brps.tile([128, 512], F32, name=f"psv{i}")
                          for i in range(4)]
                    for half in range(2):
                        for kt in range(KT):
                            for j in range(4):
                                i = 4 * half + j
                                TE.matmul(
                                    ps[j][:], w_sb["wv"][:, kt, :],
                                    xts[kt][:, i * 512:(i + 1) * 512],
                                    start=(kt == 0), stop=(kt == KT - 1))
                        if half == 0:
                            gt = enps.tile([128, B], F32, tag="gt")
                            TE.matmul(gt[0:2, :], i2e_sb[:], ech[:],
                                      start=True, stop=True)
                            V.tensor_copy(e_sb[:], gt[0:2, :])
                            nc.scalar.dma_start(
                                e_pay[:].rearrange("(p w) -> p w", p=2),
                                e_sb[:])
                            nc.gpsimd.collective_compute(
                                "AllGather", OP.bypass,
                                ins=[e_pay.opt()], outs=[e_gath.opt()],
                                replica_groups=[list(range(NCORES))])
                        for j in range(4):
                            i = 4 * half + j
                            V.bn_stats(stats[:, i, :], ps[j][:])
                            if j % 2:
                                V.tensor_copy(Y[:, i * 512:(i + 1) * 512],
                                              ps[j][:])
                            else:
                                SC.activation(Y[:, i * 512:(i + 1) * 512],
                                              ps[j][:], AF.Copy)
                    # gate math: e_gath -> own-2-head gates -> per-batch
                    # gated thresholds (exact: gate in {0,1})
                    e_all = stp.tile([H, B], F32, tag="eall")
                    nc.sync.dma_start(
                        e_all[:],
                        e_gath[:, :].rearrange("c (p w) -> (c p) w", p=2))
                    gt2 = enps.tile([128, B], F32, tag="gt")
                    TE.matmul(gt2[0:2, :], wgr2_sb[:], e_all[:],
                              start=True, stop=True)
                    gate2 = stp.tile([2, B], F32, tag="gate2")
                    V.tensor_scalar(gate2[:], gt2[0:2, :], small["bgr2"][:],
                                    0.5, OP.add, OP.is_ge)
                    gt3 = enps.tile([128, B], F32, tag="gt")
                    TE.matmul(gt3[:], i2eT_sb[:], gate2[:],
                              start=True, stop=True)
                    g128 = stp.tile([128, B], F32, tag="g128sb")
                    V.tensor_copy(g128[:], gt3[:])
                    V.tensor_scalar(thrb[:], g128[:], -2000.0,
                                    S_TH + 2000.0, OP.mult, OP.add)
                    V.tensor_scalar(biasb[:], g128[:], 1.0e6,
                                    -(1.0e6 + 128.0 * S_TH),
                                    OP.mult, OP.add)
            else:
                # weight-stationary: kt outer, 8 PSUM banks accumulate
                with tc.tile_pool(name=f"brps_{nm}", bufs=1,
                                  space="PSUM") as brps:
                    ps = [brps.tile([128, 512], F32, name=f"ps{nm}{i}")
                          for i in range(8)]
                    for kt in range(KT):
                        for nck in range(8):
                            TE.matmul(ps[nck][:], w_sb["w" + nm][:, kt, :],
                                      xts[kt][:, nck * 512:(nck + 1) * 512],
                                      start=(kt == 0), stop=(kt == KT - 1))
                    for i in range(8):
                        V.bn_stats(stats[:, i, :], ps[i][:])
                        if i % 2:
                            V.tensor_copy(Y[:, i * 512:(i + 1) * 512],
                                          ps[i][:])
                        else:
                            SC.activation(Y[:, i * 512:(i + 1) * 512],
                                          ps[i][:], AF.Copy)
            mv = stp.tile([128, 2], F32, tag="mv")
            V.bn_aggr(mv[:], stats[:])
            std = stp.tile([128, 1], F32, tag="std")
            SC.activation(std[:], mv[:, 1:2], AF.Sqrt, bias=eps_sb[:])
            thr = thr_v if nm == "v" else stp.tile([128, 1], F32,
                                                   tag=f"thr_{nm}")
            V.tensor_tensor(thr[:], std[:], small["t" + nm][:], OP.mult)
            V.tensor_tensor(thr[:], thr[:], mv[:, 0:1], OP.add)
            if nm == "v":
                continue   # v spikes happen per-batch in the attention loop
            for i in range(8):
                V.tensor_scalar(spA[nm][:, i * 512:(i + 1) * 512],
                                Y[:, i * 512:(i + 1) * 512],
                                thr[:], None, OP.is_ge)
            if nm == "k":
                for b in range(B):
                    [nc.sync, nc.scalar][b % 2].dma_start_transpose(
                        ktok[:, 8 * b:8 * b + 8, :],
                        spA["k"][:, b * NSEQ:(b + 1) * NSEQ])
                # energy elementwise part (overlaps v branch)
                GP.tensor_tensor(prod[:], spA["q"][:], spA["k"][:], OP.mult)
                V.reduce_sum(ech[:],
                             prod[:].rearrange("p (b n) -> p b n", b=B),
                             axis=mybir.AxisListType.X)

    # ============ attention -> single payload AllGather ============
    # Per batch: v spikes -> PE transpose -> KV -> S^T -> gated threshold.
    # The payload is pre-gated, so the projection after the gather uses
    # plain wp with no intermediate stage.
    with tc.tile_pool(name="atps", bufs=1, space="PSUM") as atps, \
         tc.tile_pool(name="s2ps", bufs=2, space="PSUM") as s2ps, \
         tc.tile_pool(name="tpps", bufs=2, space="PSUM") as tpps, \
         tc.tile_pool(name="kvsb", bufs=2) as kvsb:
        for b in range(B):
            n0b = b * NSEQ
            # v spikes for this batch, from the f32 copy
            V.tensor_scalar(spA["v"][:, n0b:n0b + 512],
                            Yv[:, n0b:n0b + 512], thr_v[:], None, OP.is_ge)
            V.tensor_scalar(spA["v"][:, n0b + 512:n0b + NSEQ],
                            Yv[:, n0b + 512:n0b + NSEQ], thr_v[:],
                            None, OP.is_ge)
            # PE transpose to token-major (fp16, one psum bank)
            tp_ps = tpps.tile([128, 8, 128], FP16, tag="tp")
            for j in range(8):
                TE.transpose(tp_ps[:, j, :],
                             spA["v"][:, n0b + j * 128:n0b + (j + 1) * 128],
                             idn_sb[:])
            vtok_b = kvsb.tile([128, 8, 128], FP16, tag="vtok")
            V.tensor_copy(vtok_b[:], tp_ps[:])
            # KV[b] = k_tok^T @ v_tok, heads packed in PE columns
            kv_ps = atps.tile([128, HD], F32, tag="kvps")
            for mt in range(8):
                blk = b * 8 + mt
                TE.matmul(kv_ps[0:HD, :], ktok[:, blk, 0:HD],
                          vtok_b[:, mt, 0:HD],
                          start=(mt == 0), stop=(mt == 7),
                          tile_position=(0, 0))
                TE.matmul(kv_ps[HD:128, :], ktok[:, blk, HD:128],
                          vtok_b[:, mt, HD:128],
                          start=(mt == 0), stop=(mt == 7),
                          tile_position=(0, HD))
            kv = kvsb.tile([128, HD], FP16, tag="kv")
            if b % 2:
                V.tensor_copy(kv[:], kv_ps[:])
            else:
                SC.activation(kv[:], kv_ps[:], AF.Copy)
            # S^T = KV^T @ q with gated thresholds (scores are integers;
            # saturated sigmoid stays exact, gate folds into bias/thr)
            for ncn in range(2):
                n0 = n0b + ncn * 512
                s2 = s2ps.tile([128, 512], F32, tag="s2")
                TE.matmul(s2[0:HD, :], kv[0:HD, :],
                          spA["q"][0:HD, n0:n0 + 512],
                          start=True, stop=True, tile_position=(0, 0))
                TE.matmul(s2[HD:128, :], kv[HD:128, :],
                          spA["q"][HD:128, n0:n0 + 512],
                          start=True, stop=True, tile_position=(HD, HD))
                if ncn:
                    V.tensor_scalar(payload[:, n0:n0 + 512], s2[:],
                                    thrb[:, b:b + 1], None, OP.is_ge)
                else:
                    SC.activation(payload[:, n0:n0 + 512], s2[:],
                                  AF.Sigmoid, scale=128.0,
                                  bias=biasb[:, b:b + 1])
            DENG[b % 3].dma_start(
                pay_d[:].rearrange("(p n) -> p n", p=128)[:, n0b:n0b + NSEQ],
                payload[:, n0b:n0b + NSEQ])
    nc.gpsimd.collective_compute(
        "AllGather", OP.bypass,
        ins=[pay_d.opt()], outs=[gath_d.opt()],
        replica_groups=[list(range(NCORES))])

    # ============ projection (plain wp, fp8 rhs) ============
    with tc.tile_pool(name="ppps", bufs=1, space="PSUM") as ppps, \
         tc.tile_pool(name="rhsp", bufs=1) as rhsp, \
         tc.tile_pool(name="pfin", bufs=1) as pfin:
        pp = [ppps.tile([128, 512], F32, name=f"pp{i}") for i in range(8)]
        rhs = rhsp.tile([128, KT, TOK], FP8)
        for t in range(KT):
            DENG[t % 3].dma_start(
                rhs[:, t, :],
                gath_d[t, :].rearrange("(p n) -> p n", p=128))
        for t in range(KT):
            for i in range(8):
                TE.matmul(pp[i][:], w_sb["wp"][:, t, :],
                          rhs[:, t, i * 512:(i + 1) * 512],
                          start=(t == 0), stop=(t == KT - 1))
        # ---- final BN + spike threshold + store (fp8 {0,1} exact) ----
        for i in range(8):
            V.bn_stats(pstats[:, i, :], pp[i][:])
        mv = pfin.tile([128, 2], F32)
        V.bn_aggr(mv[:], pstats[:])
        std = pfin.tile([128, 1], F32)
        SC.activation(std[:], mv[:, 1:2], AF.Sqrt, bias=eps_sb[:])
        thr = pfin.tile([128, 1], F32)
        V.tensor_tensor(thr[:], std[:], small["tp"][:], OP.mult)
        V.tensor_tensor(thr[:], thr[:], mv[:, 0:1], OP.add)
        for i in range(8):
            V.tensor_scalar(osb[:, i * 512:(i + 1) * 512],
                            pp[i][:], thr[:], None, OP.is_ge)
            if i % 2:
                DENG[(i // 2) % 3].dma_start(
                    outT.ap().rearrange("p (b n) -> p b n", b=B)[:, i // 2, :],
                    osb[:, (i - 1) * 512:(i + 1) * 512])


def _tile_rows(a):
    # (8*128, N) -> (128, 8*N) so the SBUF [p, (t n)] load is contiguous
    n = a.shape[1]
    return np.ascontiguousarray(
        a.reshape(KT, 128, n).transpose(1, 0, 2).reshape(128, KT * n))


def _prep_inputs(inputs):
    x = np.asarray(inputs["x"], np.float32)
    xT = _tile_rows(x.reshape(TOK, D).T.astype(BF))
    Wg = np.asarray(inputs["Wg"], np.float64)
    wgr = (Wg.reshape(H, HD, H).sum(axis=1).T / 1024.0).astype(np.float32)
    bg = np.asarray(inputs["bg"], np.float32)
    i2e = np.zeros((CH, 2), np.float32)
    i2e[0:HD, 0] = 1.0
    i2e[HD:CH, 1] = 1.0
    i2eT = np.ascontiguousarray(i2e.T)
    idn = np.eye(128, dtype=np.float16)
    in_maps = []
    for c in range(NCORES):
        sl = slice(CH * c, CH * c + CH)
        m = {"xT": xT, "i2e": i2e, "i2eT": i2eT, "idn": idn,
             "wgr2": np.ascontiguousarray(wgr[:, 2 * c:2 * c + 2]),
             "bgr2": np.ascontiguousarray(bg[2 * c:2 * c + 2].reshape(2, 1))}
        for nm in ("q", "k", "v", "p"):
            W = np.asarray(inputs[f"W{nm}"], np.float32)
            m["w" + nm] = _tile_rows(W[sl, :].T.astype(BF))
            g = np.asarray(inputs[f"g{nm}"], np.float32)[sl]
            be = np.asarray(inputs[f"beta{nm}"], np.float32)[sl]
            m["t" + nm] = ((2.0 - be) / g).reshape(CH, 1).astype(np.float32)
        in_maps.append(m)
    return in_maps


def _run(inputs, trace=False):
    if "nc" not in _CACHE:
        _CACHE["nc"] = _build()
    nc = _CACHE["nc"]
    in_maps = _prep_inputs(inputs)
    res = run_bass_kernel_spmd(nc, in_maps, core_ids=list(range(NCORES)),
                               trace=trace)
    out = np.empty((TOK, D), np.float32)
    for c in range(NCORES):
        out[:, CH * c:CH * c + CH] = res.results[c]["outT"].astype(np.float32).T
    return out.reshape(B, NSEQ, D), res


def kernel(**inputs) -> np.ndarray:
    out, _ = _run(inputs, trace=False)
    return out
